# revision 31
# baseline (speedup 1.0000x reference)
"""Trainium2 Bass kernel for multi-head causal attention (B=2,T=2048,C=768,H=12).

Sharding: 24 (batch, head-triple)... actually 8 shards = 2 batches x 4 head-triples.
Core c handles batch b = c//4 and heads [3*(c%4), 3*(c%4)+3).
Each core computes its heads' attention and a partial output projection
y_part.T = Wp_slice @ out_heads.T  in [C, T] layout; host sums the 4 partials
per batch and adds bp (tensor-parallel gather).

Device-side dataflow (per core), all fp32 (matmuls in float32r = full-rate fp32):
  inputs (host pre-marshalled): xqT/xkT/xvT = X[b].T [C,T], wq3/wk3 [C,192],
  wv3 [C,256] (zero-padded), wpT3 [64,3,C] = Wp.T slices.
  per t-tile j (512 wide):
    qT,kT = W.T @ X.T   (PE, contraction over C, col-packed heads)
    v     = X @ Wv      (PE, stationary xT chunks)
    scoresT[s,t] = k q.T (PE), causal-restricted columns, tri-mask on diagonal
    attnT = exp(scale*scoresT)  (ACT, PSUM->SBUF)
    outT[h] += [v|1].T @ attnT  (PE, accumulated over s-chunks; row 64 = softmax sums)
    outT[h] *= 1/sums   (DVE + gpsimd partition_broadcast)
    y.T[ci] += WpT_h @ outT[h]  (PE), DMA out
"""

import sys

sys.path.insert(0, "/opt/trn_rl_repo")

import numpy as np

import concourse.bass as bass
import concourse.tile as tile
from concourse import bacc, mybir
from concourse.bass_utils import run_bass_kernel_spmd

B, T, C = 2, 2048, 768
H, HD = 12, 64
NCORES = 8
HPC = 3  # heads per core
TT = 512  # t-tile width
SC = 128  # s-chunk width
CC = C // 128  # contraction chunks (6)
NEG = -1.0e30
F32 = mybir.dt.float32
F32R = mybir.dt.float32r
SCALE = float(HD) ** -0.5


def _r(ap):
    return ap.bitcast(F32R)


def build_nc(tT=T, trn_type=None, reps=1):
    """Build the (SPMD, core-id-free) Bass program for sequence length tT.

    reps > 1 repeats the whole computation in-NEFF (for slope timing)."""
    from contextlib import ExitStack

    nc = bacc.Bacc(
        "TRN2", target_bir_lowering=False, debug=False, num_devices=NCORES
    )
    NJ = tT // TT

    xqT = nc.dram_tensor("xqT", [C, tT], F32R, kind="ExternalInput")
    xkT = nc.dram_tensor("xkT", [C, tT], F32R, kind="ExternalInput")
    xvT = nc.dram_tensor("xvT", [C, tT], F32R, kind="ExternalInput")
    wq3 = nc.dram_tensor("wq3", [C, HPC * HD], F32R, kind="ExternalInput")
    wk3 = nc.dram_tensor("wk3", [C, HPC * HD], F32R, kind="ExternalInput")
    wv3 = nc.dram_tensor("wv3", [C, 256], F32R, kind="ExternalInput")
    wpT3 = nc.dram_tensor("wpT3", [HD, HPC, C], F32R, kind="ExternalInput")
    trimask_d = nc.dram_tensor("trimask", [128, 128], F32, kind="ExternalInput")
    yt = nc.dram_tensor("yt", [C, tT], F32, kind="ExternalOutput")

    xqT_r = xqT.rearrange("(ci p) t -> p ci t", p=128)
    xkT_r = xkT.rearrange("(ci p) t -> p ci t", p=128)
    xvT_r = xvT.rearrange("(ci p) t -> p ci t", p=128)

    NS_ALL = tT // SC  # total s-chunks

    with tile.TileContext(nc) as tc, ExitStack() as ctx:
        const = ctx.enter_context(tc.tile_pool(name="const", bufs=1))
        xpool = ctx.enter_context(tc.tile_pool(name="xt", bufs=6))
        qpool = ctx.enter_context(tc.tile_pool(name="qt", bufs=2))
        apool = ctx.enter_context(tc.tile_pool(name="attn", bufs=6))
        opool = ctx.enter_context(tc.tile_pool(name="ot", bufs=6))
        rpool = ctx.enter_context(tc.tile_pool(name="rb", bufs=6))
        ypool = ctx.enter_context(tc.tile_pool(name="y", bufs=3))
        pbig = ctx.enter_context(tc.tile_pool(name="pbig", bufs=5, space="PSUM"))
        ppo = ctx.enter_context(tc.tile_pool(name="ppo", bufs=3, space="PSUM"))

        # ---- constants / persistent tiles ----
        wq_sb = const.tile([128, CC, HPC * HD], F32R)
        wk_sb = const.tile([128, CC, HPC * HD], F32R)
        wv_sb = const.tile([128, CC, 256], F32R)
        wpT_sb = const.tile([HD, HPC, C], F32R)
        kT01 = const.tile([128, tT], F32R)
        kT2 = const.tile([HD, tT], F32R)
        vaug = const.tile([128, NS_ALL, HPC, HD + 1], F32R)
        trimask = const.tile([128, 128], F32)

        # constants go through the (otherwise idle) SWDGE/gpsimd queue so the
        # SP HWDGE queue is dedicated to streaming x.T tiles.
        nc.gpsimd.dma_start(wq_sb[:], wq3.rearrange("(ci p) w -> p ci w", p=128))
        nc.gpsimd.dma_start(wk_sb[:], wk3.rearrange("(ci p) w -> p ci w", p=128))
        nc.gpsimd.dma_start(wv_sb[:], wv3.rearrange("(ci p) w -> p ci w", p=128))
        nc.gpsimd.dma_start(wpT_sb[:], wpT3[:])
        # trimask[s', u] = 0 where u >= s' else NEG (host-precomputed)
        nc.gpsimd.dma_start(trimask[:], trimask_d[:])
        # softmax-sum ones column of v_aug (memset can't write f32r; copy-convert)
        ones_f32 = const.tile([128, max(NS_ALL * HPC, HD)], F32)
        nc.vector.memset(ones_f32[:], 1.0)
        nc.vector.tensor_copy(
            vaug[:, :, :, HD : HD + 1],
            ones_f32[:, 0 : NS_ALL * HPC]
            .rearrange("p (s h) -> p s h", h=HPC)
            .unsqueeze(3),
        )
        # gpsimd partition_broadcast (normalize) lives in the attn ucode library
        from concourse import library_config

        nc.gpsimd.load_library(library_config.attn)

        for rep, j in [(r, jj) for r in range(reps) for jj in range(NJ)]:
            tsl = slice(j * TT, (j + 1) * TT)

            # ---- load x.T tiles for this t-range ----
            xtq = xpool.tile([128, CC, TT], F32R, tag="xt", name=f"r{rep}_xtq{j}")
            nc.sync.dma_start(xtq[:], xqT_r[:, :, tsl])
            xtk = xpool.tile([128, CC, TT], F32R, tag="xt", name=f"r{rep}_xtk{j}")
            nc.sync.dma_start(xtk[:], xkT_r[:, :, tsl])
            xtv = xpool.tile([128, CC, TT], F32R, tag="xt", name=f"r{rep}_xtv{j}")
            nc.sync.dma_start(xtv[:], xvT_r[:, :, tsl])

            # ---- q/k projections: qT = Wq.T @ X.T ----
            # heads 0,1 packed in one M=128 matmul (lhsT = [Wq_h0 | Wq_h1])
            psq01 = pbig.tile([128, TT], F32, tag="blk", name=f"r{rep}_psq01_{j}")
            psq2 = pbig.tile([128, TT], F32, tag="blk", name=f"r{rep}_psq2_{j}")
            psk01 = pbig.tile([128, TT], F32, tag="blk", name=f"r{rep}_psk01_{j}")
            psk2 = pbig.tile([128, TT], F32, tag="blk", name=f"r{rep}_psk2_{j}")
            for ci in range(CC):
                st = dict(start=ci == 0, stop=ci == CC - 1, skip_group_check=True)
                nc.tensor.matmul(
                    psq01[:], wq_sb[:, ci, 0 : 2 * HD], xtq[:, ci, :], **st)
                nc.tensor.matmul(
                    psq2[0:HD, :], wq_sb[:, ci, 2 * HD : 3 * HD], xtq[:, ci, :], **st)
                nc.tensor.matmul(
                    psk01[:], wk_sb[:, ci, 0 : 2 * HD], xtk[:, ci, :], **st)
                nc.tensor.matmul(
                    psk2[0:HD, :], wk_sb[:, ci, 2 * HD : 3 * HD], xtk[:, ci, :], **st)

            qT01 = qpool.tile([128, TT], F32R, tag="q01", name=f"r{rep}_qT01_{j}")
            nc.vector.tensor_copy(qT01[:], psq01[:])
            qT2 = qpool.tile([HD, TT], F32R, tag="q2", name=f"r{rep}_qT2_{j}")
            nc.vector.tensor_copy(qT2[:], psq2[0:HD, :])
            nc.vector.tensor_copy(kT01[:, tsl], psk01[:])
            nc.vector.tensor_copy(kT2[:, tsl], psk2[0:HD, :])

            # ---- v projection (natural layout): v = X @ Wv ----
            for rr in range(TT // SC):
                si = j * (TT // SC) + rr
                psv = pbig.tile([128, 256], F32, tag="blk", name=f"r{rep}_psv_{si}")
                for ci in range(CC):
                    nc.tensor.matmul(
                        psv[:],
                        xtv[:, ci, rr * SC : (rr + 1) * SC],
                        wv_sb[:, ci, :],
                        start=ci == 0, stop=ci == CC - 1, skip_group_check=True)
                for h in range(HPC):
                    nc.vector.tensor_copy(
                        vaug[:, si, h, 0:HD], psv[:, h * HD : (h + 1) * HD])

            # ---- attention for t-tile j ----
            nsi = (j + 1) * (TT // SC)
            pso = []
            for h in range(HPC):
                pso.append(ppo.tile([128, TT], F32, tag="out", name=f"r{rep}_pso{j}_{h}"))
            for si in range(nsi):
                rr = si - j * (TT // SC)
                cols = SC * rr if rr >= 0 else 0
                ssl = slice(si * SC, (si + 1) * SC)
                for h in range(HPC):
                    if h < 2:
                        kap = kT01[HD * h : HD * h + HD, ssl]
                        qap = qT01[HD * h : HD * h + HD, cols:]
                        tp = (HD * h, 0)
                    else:
                        kap = kT2[:, ssl]
                        qap = qT2[:, cols:]
                        tp = (0, 0)
                    pss = pbig.tile([128, TT], F32, tag="blk", name=f"r{rep}_pss{j}_{si}_{h}")
                    nc.tensor.matmul(
                        pss[:, cols:], kap, qap,
                        start=True, stop=True, skip_group_check=True,
                        tile_position=tp)
                    if rr >= 0:
                        nc.vector.tensor_add(
                            pss[:, cols : cols + SC], pss[:, cols : cols + SC],
                            trimask[:])
                    at = apool.tile([128, TT], F32R, tag="at", name=f"r{rep}_at{j}_{si}_{h}")
                    nc.scalar.activation(
                        at[:, cols:], pss[:, cols:],
                        mybir.ActivationFunctionType.Exp, scale=SCALE)
                    nc.tensor.matmul(
                        pso[h][0 : HD + 1, cols:],
                        vaug[:, si, h, :],
                        at[:, cols:],
                        start=si == 0, stop=si == nsi - 1, skip_group_check=True)

            # ---- normalize: outT[h] = pso[h][0:64] / pso[h][64] ----
            ots = []
            for h in range(HPC):
                rb = rpool.tile([128, TT], F32, tag="rb", name=f"r{rep}_rb{j}_{h}")
                nc.vector.reciprocal(rb[HD : HD + 1, :], pso[h][HD : HD + 1, :])
                # broadcast row 64 to partitions 0..63 on the idle gpsimd engine
                nc.gpsimd.partition_broadcast(rb[0:HD, :], rb[HD : HD + 1, :])
                ot = opool.tile([HD, TT], F32R, tag="ot", name=f"r{rep}_ot{j}_{h}")
                nc.vector.tensor_mul(ot[:], pso[h][0:HD, :], rb[0:HD, :])
                ots.append(ot)

            # ---- output projection: y.T[ci] = sum_h WpT_h @ outT_h ----
            for ci in range(CC):
                psy = pbig.tile([128, TT], F32, tag="blk", name=f"r{rep}_psy{j}_{ci}")
                for h in range(HPC):
                    nc.tensor.matmul(
                        psy[:],
                        wpT_sb[:, h, ci * 128 : (ci + 1) * 128],
                        ots[h][:],
                        start=h == 0, stop=h == HPC - 1, skip_group_check=True)
                ysb = ypool.tile([128, TT], F32, tag="y", name=f"r{rep}_ysb{j}_{ci}")
                nc.vector.tensor_copy(ysb[:], psy[:])
                # stores go through the SWDGE/gpsimd queue (SP streams loads)
                nc.gpsimd.dma_start(yt[ci * 128 : (ci + 1) * 128, tsl], ysb[:])

    if not nc.is_finalized():
        nc.finalize()
    return nc


_CACHE = {}


def _get_nc(tT=T):
    if tT not in _CACHE:
        _CACHE[tT] = build_nc(tT)
    return _CACHE[tT]


def round_f32r(a):
    """Round fp32 array to fp32r (8-bit exp, 11-bit mantissa, RNE) bit pattern."""
    u = np.ascontiguousarray(a, np.float32).view(np.uint32).astype(np.uint64)
    lsb = (u >> 12) & 1
    r = (u + 0x7FF + lsb) & 0xFFFFF000
    # saturate overflow past inf is impossible for our magnitudes; just mask
    return (r & 0xFFFFFFFF).astype(np.uint32).view(np.float32)


def make_in_maps(query, key, value, Wq, Wk, Wv, Wp, tT=T):
    query = np.asarray(query, np.float32)
    key = np.asarray(key, np.float32)
    value = np.asarray(value, np.float32)
    Wq = np.asarray(Wq, np.float32)
    Wk = np.asarray(Wk, np.float32)
    Wv = np.asarray(Wv, np.float32)
    Wp = np.asarray(Wp, np.float32)
    # trimask[s', u] = 0 where u >= s' else NEG
    trimask = np.where(
        np.arange(128)[None, :] >= np.arange(128)[:, None], 0.0, NEG
    ).astype(np.float32)
    in_maps = []
    for core in range(NCORES):
        b = core // (NCORES // B)
        h0 = HPC * (core % (NCORES // B))
        wq = Wq[h0 : h0 + HPC].transpose(1, 0, 2).reshape(C, HPC * HD)
        wk = Wk[h0 : h0 + HPC].transpose(1, 0, 2).reshape(C, HPC * HD)
        wv = np.zeros((C, 256), np.float32)
        wv[:, : HPC * HD] = Wv[h0 : h0 + HPC].transpose(1, 0, 2).reshape(C, HPC * HD)
        wpT = (
            Wp[:, h0 * HD : (h0 + HPC) * HD].T
            .reshape(HPC, HD, C)
            .transpose(1, 0, 2)
        )
        in_maps.append(
            {
                "xqT": round_f32r(query[b, :tT].T),
                "xkT": round_f32r(key[b, :tT].T),
                "xvT": round_f32r(value[b, :tT].T),
                "wq3": round_f32r(wq),
                "wk3": round_f32r(wk),
                "wv3": round_f32r(wv),
                "wpT3": round_f32r(wpT),
                "trimask": trimask,
            }
        )
    return in_maps


def gather_output(results, bp, tT=T):
    bp = np.asarray(bp, np.float32)
    gpb = NCORES // B  # cores per batch
    y = np.empty((B, tT, C), np.float32)
    for b in range(B):
        acc = results[gpb * b]["yt"].copy()
        for c in range(1, gpb):
            acc += results[gpb * b + c]["yt"]
        y[b] = acc.T + bp
    return y


def _make_runner(nc, n_cores):
    """Compile-once runner (run_bass_kernel_spmd re-jits per call)."""
    import jax
    from jax.sharding import Mesh, PartitionSpec
    from jax.experimental.shard_map import shard_map
    from concourse import bass2jax

    bass2jax.install_neuronx_cc_hook()
    partition_name = nc.partition_id_tensor.name if nc.partition_id_tensor else None
    in_names, out_names, out_avals, zero_outs = [], [], [], []
    for alloc in nc.m.functions[0].allocations:
        if not isinstance(alloc, mybir.MemoryLocationSet):
            continue
        name = alloc.memorylocations[0].name
        if alloc.kind == "ExternalInput":
            if name != partition_name:
                in_names.append(name)
        elif alloc.kind == "ExternalOutput":
            out_names.append(name)
            shape = tuple(alloc.tensor_shape)
            dtype = mybir.dt.np(alloc.dtype)
            out_avals.append(jax.core.ShapedArray(shape, dtype))
            zero_outs.append(np.zeros(shape, dtype))
    n_params = len(in_names)
    n_outs = len(out_avals)
    all_in_names = list(in_names) + list(out_names)
    if partition_name is not None:
        all_in_names.append(partition_name)

    def _body(*args):
        operands = list(args)
        if partition_name is not None:
            operands.append(bass2jax.partition_id_tensor())
        outs = bass2jax._bass_exec_p.bind(
            *operands,
            out_avals=tuple(out_avals),
            in_names=tuple(all_in_names),
            out_names=tuple(out_names),
            lowering_input_output_aliases=(),
            sim_require_finite=True,
            sim_require_nnan=True,
            nc=nc,
        )
        return tuple(outs)

    devices = jax.devices()[:n_cores]
    mesh = Mesh(np.asarray(devices), ("core",))
    in_specs = (PartitionSpec("core"),) * (n_params + n_outs)
    out_specs = (PartitionSpec("core"),) * n_outs
    fn = jax.jit(
        shard_map(_body, mesh=mesh, in_specs=in_specs, out_specs=out_specs,
                  check_rep=False),
        donate_argnums=tuple(range(n_params, n_params + n_outs)),
        keep_unused=True)

    def run(in_maps):
        per_core = [[np.asarray(m[nm]) for nm in in_names] for m in in_maps]
        concat_in = [
            np.concatenate([per_core[c][i] for c in range(n_cores)], axis=0)
            for i in range(n_params)
        ]
        concat_zeros = [
            np.zeros((n_cores * z.shape[0], *z.shape[1:]), z.dtype)
            for z in zero_outs
        ]
        out_arrs = fn(*concat_in, *concat_zeros)
        return [
            {
                name: np.asarray(out_arrs[i]).reshape(
                    n_cores, *out_avals[i].shape)[c]
                for i, name in enumerate(out_names)
            }
            for c in range(n_cores)
        ]

    return run


def _get_runner(tT=T):
    key = ("runner", tT)
    if key not in _CACHE:
        _CACHE[key] = _make_runner(_get_nc(tT), NCORES)
    return _CACHE[key]


def kernel(query, key, value, Wq, Wk, Wv, Wp, bp):
    in_maps = make_in_maps(query, key, value, Wq, Wk, Wv, Wp)
    results = _get_runner()(in_maps)
    return gather_output(results, bp)


# revision 32
# speedup vs baseline: 209.3802x; 209.3802x over previous
"""Trainium2 Bass kernel for multi-head causal attention (B=2,T=2048,C=768,H=12).

Sharding: 24 (batch, head-triple)... actually 8 shards = 2 batches x 4 head-triples.
Core c handles batch b = c//4 and heads [3*(c%4), 3*(c%4)+3).
Each core computes its heads' attention and a partial output projection
y_part.T = Wp_slice @ out_heads.T  in [C, T] layout; host sums the 4 partials
per batch and adds bp (tensor-parallel gather).

Device-side dataflow (per core), all fp32 (matmuls in float32r = full-rate fp32):
  inputs (host pre-marshalled): xqT/xkT/xvT = X[b].T [C,T], wq3/wk3 [C,192],
  wv3 [C,256] (zero-padded), wpT3 [64,3,C] = Wp.T slices.
  per t-tile j (512 wide):
    qT,kT = W.T @ X.T   (PE, contraction over C, col-packed heads)
    v     = X @ Wv      (PE, stationary xT chunks)
    scoresT[s,t] = k q.T (PE), causal-restricted columns, tri-mask on diagonal
    attnT = exp(scale*scoresT)  (ACT, PSUM->SBUF)
    outT[h] += [v|1].T @ attnT  (PE, accumulated over s-chunks; row 64 = softmax sums)
    outT[h] *= 1/sums   (DVE + gpsimd partition_broadcast)
    y.T[ci] += WpT_h @ outT[h]  (PE), DMA out
"""

import sys

sys.path.insert(0, "/opt/trn_rl_repo")

import numpy as np

import concourse.bass as bass
import concourse.tile as tile
from concourse import bacc, mybir
from concourse.bass_utils import run_bass_kernel_spmd

B, T, C = 2, 2048, 768
H, HD = 12, 64
NCORES = 8
HPC = 3  # heads per core
TT = 512  # t-tile width
SC = 128  # s-chunk width
CC = C // 128  # contraction chunks (6)
NEG = -1.0e30
F32 = mybir.dt.float32
F32R = mybir.dt.float32r
SCALE = float(HD) ** -0.5


def _r(ap):
    return ap.bitcast(F32R)


def build_nc(tT=T, trn_type=None, reps=1, loop_reps=0):
    """Build the (SPMD, core-id-free) Bass program for sequence length tT.

    reps > 1 statically repeats the computation in-NEFF; loop_reps > 0 wraps
    the body in a hardware For_i loop instead (both for slope timing)."""
    from contextlib import ExitStack

    nc = bacc.Bacc(
        "TRN2", target_bir_lowering=False, debug=False, num_devices=NCORES
    )
    NJ = tT // TT

    xqT = nc.dram_tensor("xqT", [C, tT], F32R, kind="ExternalInput")
    xkT = nc.dram_tensor("xkT", [C, tT], F32R, kind="ExternalInput")
    xvT = nc.dram_tensor("xvT", [C, tT], F32R, kind="ExternalInput")
    wq3 = nc.dram_tensor("wq3", [C, HPC * HD], F32R, kind="ExternalInput")
    wk3 = nc.dram_tensor("wk3", [C, HPC * HD], F32R, kind="ExternalInput")
    wv3 = nc.dram_tensor("wv3", [C, 256], F32R, kind="ExternalInput")
    wpT3 = nc.dram_tensor("wpT3", [HD, HPC, C], F32R, kind="ExternalInput")
    trimask_d = nc.dram_tensor("trimask", [128, 128], F32, kind="ExternalInput")
    yt = nc.dram_tensor("yt", [C, tT], F32, kind="ExternalOutput")

    xqT_r = xqT.rearrange("(ci p) t -> p ci t", p=128)
    xkT_r = xkT.rearrange("(ci p) t -> p ci t", p=128)
    xvT_r = xvT.rearrange("(ci p) t -> p ci t", p=128)

    NS_ALL = tT // SC  # total s-chunks

    with tile.TileContext(nc) as tc, ExitStack() as ctx:
        const = ctx.enter_context(tc.tile_pool(name="const", bufs=1))
        xpool = ctx.enter_context(tc.tile_pool(name="xt", bufs=6))
        qpool = ctx.enter_context(tc.tile_pool(name="qt", bufs=2))
        apool = ctx.enter_context(tc.tile_pool(name="attn", bufs=6))
        opool = ctx.enter_context(tc.tile_pool(name="ot", bufs=6))
        rpool = ctx.enter_context(tc.tile_pool(name="rb", bufs=6))
        ypool = ctx.enter_context(tc.tile_pool(name="y", bufs=3))
        pbig = ctx.enter_context(tc.tile_pool(name="pbig", bufs=5, space="PSUM"))
        ppo = ctx.enter_context(tc.tile_pool(name="ppo", bufs=3, space="PSUM"))

        # ---- constants / persistent tiles ----
        wq_sb = const.tile([128, CC, HPC * HD], F32R)
        wk_sb = const.tile([128, CC, HPC * HD], F32R)
        wv_sb = const.tile([128, CC, 256], F32R)
        wpT_sb = const.tile([HD, HPC, C], F32R)
        kT01 = const.tile([128, tT], F32R)
        kT2 = const.tile([HD, tT], F32R)
        vaug = const.tile([128, NS_ALL, HPC, HD + 1], F32R)
        trimask = const.tile([128, 128], F32)

        # constants go through the (otherwise idle) SWDGE/gpsimd queue so the
        # SP HWDGE queue is dedicated to streaming x.T tiles.
        nc.gpsimd.dma_start(wq_sb[:], wq3.rearrange("(ci p) w -> p ci w", p=128))
        nc.gpsimd.dma_start(wk_sb[:], wk3.rearrange("(ci p) w -> p ci w", p=128))
        nc.gpsimd.dma_start(wv_sb[:], wv3.rearrange("(ci p) w -> p ci w", p=128))
        nc.gpsimd.dma_start(wpT_sb[:], wpT3[:])
        # trimask[s', u] = 0 where u >= s' else NEG (host-precomputed)
        nc.gpsimd.dma_start(trimask[:], trimask_d[:])
        # softmax-sum ones column of v_aug (memset can't write f32r; copy-convert)
        ones_f32 = const.tile([128, max(NS_ALL * HPC, HD)], F32)
        nc.vector.memset(ones_f32[:], 1.0)
        nc.vector.tensor_copy(
            vaug[:, :, :, HD : HD + 1],
            ones_f32[:, 0 : NS_ALL * HPC]
            .rearrange("p (s h) -> p s h", h=HPC)
            .unsqueeze(3),
        )
        # gpsimd partition_broadcast (normalize) lives in the attn ucode library
        from concourse import library_config

        nc.gpsimd.load_library(library_config.attn)

        loop_cm = tc.For_i(0, loop_reps, 1) if loop_reps else None
        if loop_cm is not None:
            loop_cm.__enter__()
        for rep, j in [(r, jj) for r in range(reps) for jj in range(NJ)]:
            tsl = slice(j * TT, (j + 1) * TT)

            # ---- load x.T tiles for this t-range ----
            xtq = xpool.tile([128, CC, TT], F32R, tag="xt", name=f"r{rep}_xtq{j}")
            nc.sync.dma_start(xtq[:], xqT_r[:, :, tsl])
            xtk = xpool.tile([128, CC, TT], F32R, tag="xt", name=f"r{rep}_xtk{j}")
            nc.sync.dma_start(xtk[:], xkT_r[:, :, tsl])
            xtv = xpool.tile([128, CC, TT], F32R, tag="xt", name=f"r{rep}_xtv{j}")
            nc.sync.dma_start(xtv[:], xvT_r[:, :, tsl])

            # ---- q/k projections: qT = Wq.T @ X.T ----
            # heads 0,1 packed in one M=128 matmul (lhsT = [Wq_h0 | Wq_h1])
            psq01 = pbig.tile([128, TT], F32, tag="blk", name=f"r{rep}_psq01_{j}")
            psq2 = pbig.tile([128, TT], F32, tag="blk", name=f"r{rep}_psq2_{j}")
            psk01 = pbig.tile([128, TT], F32, tag="blk", name=f"r{rep}_psk01_{j}")
            psk2 = pbig.tile([128, TT], F32, tag="blk", name=f"r{rep}_psk2_{j}")
            for ci in range(CC):
                st = dict(start=ci == 0, stop=ci == CC - 1, skip_group_check=True)
                nc.tensor.matmul(
                    psq01[:], wq_sb[:, ci, 0 : 2 * HD], xtq[:, ci, :], **st)
                nc.tensor.matmul(
                    psq2[0:HD, :], wq_sb[:, ci, 2 * HD : 3 * HD], xtq[:, ci, :], **st)
                nc.tensor.matmul(
                    psk01[:], wk_sb[:, ci, 0 : 2 * HD], xtk[:, ci, :], **st)
                nc.tensor.matmul(
                    psk2[0:HD, :], wk_sb[:, ci, 2 * HD : 3 * HD], xtk[:, ci, :], **st)

            qT01 = qpool.tile([128, TT], F32R, tag="q01", name=f"r{rep}_qT01_{j}")
            nc.vector.tensor_copy(qT01[:], psq01[:])
            qT2 = qpool.tile([HD, TT], F32R, tag="q2", name=f"r{rep}_qT2_{j}")
            nc.vector.tensor_copy(qT2[:], psq2[0:HD, :])
            nc.vector.tensor_copy(kT01[:, tsl], psk01[:])
            nc.vector.tensor_copy(kT2[:, tsl], psk2[0:HD, :])

            # ---- v projection (natural layout): v = X @ Wv ----
            for rr in range(TT // SC):
                si = j * (TT // SC) + rr
                psv = pbig.tile([128, 256], F32, tag="blk", name=f"r{rep}_psv_{si}")
                for ci in range(CC):
                    nc.tensor.matmul(
                        psv[:],
                        xtv[:, ci, rr * SC : (rr + 1) * SC],
                        wv_sb[:, ci, :],
                        start=ci == 0, stop=ci == CC - 1, skip_group_check=True)
                for h in range(HPC):
                    nc.vector.tensor_copy(
                        vaug[:, si, h, 0:HD], psv[:, h * HD : (h + 1) * HD])

            # ---- attention for t-tile j ----
            nsi = (j + 1) * (TT // SC)
            pso = []
            for h in range(HPC):
                pso.append(ppo.tile([128, TT], F32, tag="out", name=f"r{rep}_pso{j}_{h}"))
            for si in range(nsi):
                rr = si - j * (TT // SC)
                cols = SC * rr if rr >= 0 else 0
                ssl = slice(si * SC, (si + 1) * SC)
                for h in range(HPC):
                    if h < 2:
                        kap = kT01[HD * h : HD * h + HD, ssl]
                        qap = qT01[HD * h : HD * h + HD, cols:]
                        tp = (HD * h, 0)
                    else:
                        kap = kT2[:, ssl]
                        qap = qT2[:, cols:]
                        tp = (0, 0)
                    pss = pbig.tile([128, TT], F32, tag="blk", name=f"r{rep}_pss{j}_{si}_{h}")
                    nc.tensor.matmul(
                        pss[:, cols:], kap, qap,
                        start=True, stop=True, skip_group_check=True,
                        tile_position=tp)
                    if rr >= 0:
                        nc.vector.tensor_add(
                            pss[:, cols : cols + SC], pss[:, cols : cols + SC],
                            trimask[:])
                    at = apool.tile([128, TT], F32R, tag="at", name=f"r{rep}_at{j}_{si}_{h}")
                    nc.scalar.activation(
                        at[:, cols:], pss[:, cols:],
                        mybir.ActivationFunctionType.Exp, scale=SCALE)
                    nc.tensor.matmul(
                        pso[h][0 : HD + 1, cols:],
                        vaug[:, si, h, :],
                        at[:, cols:],
                        start=si == 0, stop=si == nsi - 1, skip_group_check=True)

            # ---- normalize: outT[h] = pso[h][0:64] / pso[h][64] ----
            ots = []
            for h in range(HPC):
                rb = rpool.tile([128, TT], F32, tag="rb", name=f"r{rep}_rb{j}_{h}")
                nc.vector.reciprocal(rb[HD : HD + 1, :], pso[h][HD : HD + 1, :])
                # broadcast row 64 to partitions 0..63 on the idle gpsimd engine
                nc.gpsimd.partition_broadcast(rb[0:HD, :], rb[HD : HD + 1, :])
                ot = opool.tile([HD, TT], F32R, tag="ot", name=f"r{rep}_ot{j}_{h}")
                nc.vector.tensor_mul(ot[:], pso[h][0:HD, :], rb[0:HD, :])
                ots.append(ot)

            # ---- output projection: y.T[ci] = sum_h WpT_h @ outT_h ----
            for ci in range(CC):
                psy = pbig.tile([128, TT], F32, tag="blk", name=f"r{rep}_psy{j}_{ci}")
                for h in range(HPC):
                    nc.tensor.matmul(
                        psy[:],
                        wpT_sb[:, h, ci * 128 : (ci + 1) * 128],
                        ots[h][:],
                        start=h == 0, stop=h == HPC - 1, skip_group_check=True)
                ysb = ypool.tile([128, TT], F32, tag="y", name=f"r{rep}_ysb{j}_{ci}")
                nc.vector.tensor_copy(ysb[:], psy[:])
                # stores go through the SWDGE/gpsimd queue (SP streams loads)
                nc.gpsimd.dma_start(yt[ci * 128 : (ci + 1) * 128, tsl], ysb[:])

        if loop_cm is not None:
            loop_cm.__exit__(None, None, None)

    if not nc.is_finalized():
        nc.finalize()
    return nc


_CACHE = {}


def _get_nc(tT=T):
    if tT not in _CACHE:
        _CACHE[tT] = build_nc(tT)
    return _CACHE[tT]


def round_f32r(a):
    """Round fp32 array to fp32r (8-bit exp, 11-bit mantissa, RNE) bit pattern."""
    u = np.ascontiguousarray(a, np.float32).view(np.uint32).astype(np.uint64)
    lsb = (u >> 12) & 1
    r = (u + 0x7FF + lsb) & 0xFFFFF000
    # saturate overflow past inf is impossible for our magnitudes; just mask
    return (r & 0xFFFFFFFF).astype(np.uint32).view(np.float32)


def make_in_maps(query, key, value, Wq, Wk, Wv, Wp, tT=T):
    query = np.asarray(query, np.float32)
    key = np.asarray(key, np.float32)
    value = np.asarray(value, np.float32)
    Wq = np.asarray(Wq, np.float32)
    Wk = np.asarray(Wk, np.float32)
    Wv = np.asarray(Wv, np.float32)
    Wp = np.asarray(Wp, np.float32)
    # trimask[s', u] = 0 where u >= s' else NEG
    trimask = np.where(
        np.arange(128)[None, :] >= np.arange(128)[:, None], 0.0, NEG
    ).astype(np.float32)
    in_maps = []
    for core in range(NCORES):
        b = core // (NCORES // B)
        h0 = HPC * (core % (NCORES // B))
        wq = Wq[h0 : h0 + HPC].transpose(1, 0, 2).reshape(C, HPC * HD)
        wk = Wk[h0 : h0 + HPC].transpose(1, 0, 2).reshape(C, HPC * HD)
        wv = np.zeros((C, 256), np.float32)
        wv[:, : HPC * HD] = Wv[h0 : h0 + HPC].transpose(1, 0, 2).reshape(C, HPC * HD)
        wpT = (
            Wp[:, h0 * HD : (h0 + HPC) * HD].T
            .reshape(HPC, HD, C)
            .transpose(1, 0, 2)
        )
        in_maps.append(
            {
                "xqT": round_f32r(query[b, :tT].T),
                "xkT": round_f32r(key[b, :tT].T),
                "xvT": round_f32r(value[b, :tT].T),
                "wq3": round_f32r(wq),
                "wk3": round_f32r(wk),
                "wv3": round_f32r(wv),
                "wpT3": round_f32r(wpT),
                "trimask": trimask,
            }
        )
    return in_maps


def gather_output(results, bp, tT=T):
    bp = np.asarray(bp, np.float32)
    gpb = NCORES // B  # cores per batch
    y = np.empty((B, tT, C), np.float32)
    for b in range(B):
        acc = results[gpb * b]["yt"].copy()
        for c in range(1, gpb):
            acc += results[gpb * b + c]["yt"]
        y[b] = acc.T + bp
    return y


def _make_runner(nc, n_cores):
    """Compile-once runner (run_bass_kernel_spmd re-jits per call)."""
    import jax
    from jax.sharding import Mesh, PartitionSpec
    from jax.experimental.shard_map import shard_map
    from concourse import bass2jax

    bass2jax.install_neuronx_cc_hook()
    partition_name = nc.partition_id_tensor.name if nc.partition_id_tensor else None
    in_names, out_names, out_avals, zero_outs = [], [], [], []
    for alloc in nc.m.functions[0].allocations:
        if not isinstance(alloc, mybir.MemoryLocationSet):
            continue
        name = alloc.memorylocations[0].name
        if alloc.kind == "ExternalInput":
            if name != partition_name:
                in_names.append(name)
        elif alloc.kind == "ExternalOutput":
            out_names.append(name)
            shape = tuple(alloc.tensor_shape)
            dtype = mybir.dt.np(alloc.dtype)
            out_avals.append(jax.core.ShapedArray(shape, dtype))
            zero_outs.append(np.zeros(shape, dtype))
    n_params = len(in_names)
    n_outs = len(out_avals)
    all_in_names = list(in_names) + list(out_names)
    if partition_name is not None:
        all_in_names.append(partition_name)

    def _body(*args):
        operands = list(args)
        if partition_name is not None:
            operands.append(bass2jax.partition_id_tensor())
        outs = bass2jax._bass_exec_p.bind(
            *operands,
            out_avals=tuple(out_avals),
            in_names=tuple(all_in_names),
            out_names=tuple(out_names),
            lowering_input_output_aliases=(),
            sim_require_finite=True,
            sim_require_nnan=True,
            nc=nc,
        )
        return tuple(outs)

    devices = jax.devices()[:n_cores]
    mesh = Mesh(np.asarray(devices), ("core",))
    in_specs = (PartitionSpec("core"),) * (n_params + n_outs)
    out_specs = (PartitionSpec("core"),) * n_outs
    fn = jax.jit(
        shard_map(_body, mesh=mesh, in_specs=in_specs, out_specs=out_specs,
                  check_rep=False),
        donate_argnums=tuple(range(n_params, n_params + n_outs)),
        keep_unused=True)

    def run(in_maps):
        per_core = [[np.asarray(m[nm]) for nm in in_names] for m in in_maps]
        concat_in = [
            np.concatenate([per_core[c][i] for c in range(n_cores)], axis=0)
            for i in range(n_params)
        ]
        concat_zeros = [
            np.zeros((n_cores * z.shape[0], *z.shape[1:]), z.dtype)
            for z in zero_outs
        ]
        out_arrs = fn(*concat_in, *concat_zeros)
        return [
            {
                name: np.asarray(out_arrs[i]).reshape(
                    n_cores, *out_avals[i].shape)[c]
                for i, name in enumerate(out_names)
            }
            for c in range(n_cores)
        ]

    return run


def _get_runner(tT=T):
    key = ("runner", tT)
    if key not in _CACHE:
        _CACHE[key] = _make_runner(_get_nc(tT), NCORES)
    return _CACHE[key]


def kernel(query, key, value, Wq, Wk, Wv, Wp, bp):
    in_maps = make_in_maps(query, key, value, Wq, Wk, Wv, Wp)
    results = _get_runner()(in_maps)
    return gather_output(results, bp)
